# revision 70
# baseline (speedup 1.0000x reference)
"""Trainium2 Bass kernel for nn_MultiHeadAttention_48395691492101.

Strategy: pure head-parallel sharding across 8 NeuronCores (2 heads/core).
Because the reference reshapes ctx [B,H,T,DV] -> [B,T,H*DV] WITHOUT
transposing, row-block t' in [h*128,(h+1)*128) of the reshaped tensor comes
entirely from head h.  Core c (heads 2c,2c+1) therefore owns output rows
[c*256,(c+1)*256) of every batch, and the output projection needs no
cross-core reduction at all.

Datapath (fp32 PSUM accumulation everywhere):
- QKV projections: residual-compensated fp8e4m3 DoubleRow (x8*w8 + xr*w8
  + x8*wr, weights pre-scaled x16 out of the subnormal range) - half the
  bf16 PE cost at ~bf16 accuracy.
- scores: plain fp8 DoubleRow on post-projection-quantized Q/K (softmax
  damps bounded post-quantization error to ~9e-3); Q/K fold [128,T] ->
  [64,2,T] via per-t-group SBUF->SBUF DMAs on the sync queue.
- causal: dead 128x512 blocks skipped; diagonal blocks get the triangular
  mask via an accumulating id^T@dmask matmul on the PE.  PSUM start=True
  lazily zero-marks its WHOLE 2KB bank region, so per region exactly one
  matmul carries start=True and spans are ordered marker-first.
- PV is q-partitioned: out [128 q, 65] per (head, q-chunk) with the at
  block as stationary - 65 moving columns instead of 512 halves the PE
  cost of PV; a vnb ones-column yields softmax denominators per
  partition, normalization is a per-partition DVE reciprocal +
  tensor_scalar multiply straight out of PSUM (no gpsimd broadcast), and
  a paired PE transpose + strided copies build the Y^T gather.  The 8
  accumulators live as 512B slots in one [128,8,128] psum tile.
- out-projection: bf16 (fp8 here busts the 2e-2 error budget - operand
  quantization goes Gaussian through the 1024-long contraction).
- flat software-pipelined emission: every chunk's PV is emitted only
  after the next chunk's scores; the q-group close is split (DVE-only
  normalize immediately, transpose+copies spread over the following 4
  flushes) so the PE stream never waits on DVE; boundary fillers (next
  batch's QKV, V transposes, the previous batch's output projection)
  keep the PE fed across q-group seams.  Output stores are emitted one
  batch late; a warmup matmul chain rides the PE p-state ramp.
"""

import sys

if "/opt/trn_rl_repo" not in sys.path:
    sys.path.insert(0, "/opt/trn_rl_repo")

import numpy as np
import ml_dtypes

BF16 = ml_dtypes.bfloat16
FP8E4 = ml_dtypes.float8_e4m3

B, T, D = 4, 2048, 1024
H, DK, DV = 16, 64, 64
SCALE = np.float32(1.0 / 8.0)
NCORES = 8
HP = H // NCORES          # heads per core = 2
ROWS = HP * (T * DV) // D  # output rows per head pair per batch = 256
NDC = D // 128            # 8 d-chunks
NTG = 4                   # t-groups of 512 for QKV
NQG = 4                   # q-groups of 512
NKC = T // 128            # 16 k-chunks
MASK_NEG = np.float32(-8.0e9)   # becomes -1e9 after *SCALE inside exp

# moving-operand dtype knobs: f32r avoids Ldweights instructions on the PE
# sequencer; bf16 halves DMA and is 1 cycle/row at any output width
XT_F32R = False
QT_F32R = False
AT_F32R = False
WOUT_F32R = False
XT_BUFS = 4
USE_FP8 = True   # fp8e4m3 + DoubleRow for QKV projections and score matmuls
OUT_FP8 = False  # fp8 out-projection busts the error budget (ctx/wout
                 # quantization goes Gaussian through the 1024-long
                 # contraction); bf16 out-proj stays
# Residual-compensated fp8 QKV projections: Q = x8*w8 + xr*w8 + x8*wr with
# xr = fp8(x - x8), wr = fp8(WSC*w - w8).  The residuals live in e4m3's
# subnormal range, which is fine (absolute precision is what residuals
# need; verified exact on HW).  3 DoubleRow chains = half the bf16 cost
# with ~bf16 accuracy.  Score matmuls quantize Q/K *post*-projection,
# which softmax damps (~9e-3), so they stay plain fp8.
QKV_COMP = True
# fp8 weight pre-scale: the projection weights have std ~0.02, deep in
# e4m3's subnormal range where quantization error turns absolute.  Scaling
# by 16 moves them into the normal range (uniform ~4% rel err); the factor
# is folded out exactly via power-of-2 rescales of the biases, the exp
# scale, and the dropout mask.
WSC = 16.0                       # weight scale (wq/wk/wv and wout)
SSC = WSC * WSC                  # score scale (q*k both carry WSC)
YSC = WSC if USE_FP8 else 1.0    # scale carried by yts (ctx) via V
OSC = YSC * (WSC if OUT_FP8 else 1.0)  # scale of the out-proj PSUM

_cache = {}


def _build(causal: bool, debug: bool = False):
    import concourse.tile as tile
    import concourse.mybir as mybir
    from concourse import bacc

    F32 = mybir.dt.float32
    F32R = mybir.dt.float32r
    BF = mybir.dt.bfloat16
    FP8 = mybir.dt.float8e4
    XT_DT = FP8 if USE_FP8 else (F32R if XT_F32R else BF)   # pairs with wq/wk/wv
    QT_DT = FP8 if USE_FP8 else (F32R if QT_F32R else BF)   # pairs with kt
    AT_DT = F32R if AT_F32R else BF      # pairs with vnb (and vt/id/tp chain)
    WOUT_DT = FP8 if OUT_FP8 else (F32R if WOUT_F32R else BF)  # pairs with yts
    W_DT = XT_DT
    KT_DT = QT_DT
    VNB_DT = AT_DT
    VT_DT = AT_DT
    ID_DT = AT_DT
    YT_DT = WOUT_DT
    Exp = mybir.ActivationFunctionType.Exp

    nc = bacc.Bacc("TRN2", target_bir_lowering=False, debug=False,
                   num_devices=NCORES)

    # host-prearranged layouts:
    #   fp8: xT [128, c2, i, B*T] with d = c2*256 + 2p + i, w* [128, (c2 i j)]
    #   else: xT [128, dc, B*T] with d = dc*128 + p, w* [128, (dc j)]
    if USE_FP8:
        xT_d = nc.dram_tensor("xT", [128, 4, 2, B * T], XT_DT,
                              kind="ExternalInput").ap()
        if QKV_COMP:
            xTr_d = nc.dram_tensor("xTr", [128, 4, 2, B * T], XT_DT,
                                   kind="ExternalInput").ap()
    else:
        xT_d = nc.dram_tensor("xT", [128, NDC, B * T], XT_DT,
                              kind="ExternalInput").ap()
    wq_d = nc.dram_tensor("wq", [128, D], W_DT, kind="ExternalInput").ap()
    wk_d = nc.dram_tensor("wk", [128, D], W_DT, kind="ExternalInput").ap()
    wv_d = nc.dram_tensor("wv", [128, D], W_DT, kind="ExternalInput").ap()
    if USE_FP8 and QKV_COMP:
        wqr_d = nc.dram_tensor("wqr", [128, D], W_DT, kind="ExternalInput").ap()
        wkr_d = nc.dram_tensor("wkr", [128, D], W_DT, kind="ExternalInput").ap()
        wvr_d = nc.dram_tensor("wvr", [128, D], W_DT, kind="ExternalInput").ap()
    bq_d = nc.dram_tensor("bq", [128, 1], F32, kind="ExternalInput").ap()
    bk_d = nc.dram_tensor("bk", [128, 1], F32, kind="ExternalInput").ap()
    bv_d = nc.dram_tensor("bv", [128, 1], F32, kind="ExternalInput").ap()
    wout_d = nc.dram_tensor("wout", [128, NDC * D], WOUT_DT, kind="ExternalInput").ap()
    bout_d = nc.dram_tensor("bout", [1, D], F32R, kind="ExternalInput").ap()
    onesr_d = nc.dram_tensor("onesr", [1, 128], F32R, kind="ExternalInput").ap()
    drop_d = nc.dram_tensor("drop", [B, ROWS, D], BF, kind="ExternalInput").ap()
    id_d = nc.dram_tensor("idm", [128, 128], ID_DT, kind="ExternalInput").ap()
    if causal:
        dmask_d = nc.dram_tensor("dmask", [128, 128], AT_DT,
                                 kind="ExternalInput").ap()
    else:
        maskT_d = nc.dram_tensor("maskT", [T, T], F32, kind="ExternalInput").ap()
    out_d = nc.dram_tensor("out", [B, ROWS, D], F32, kind="ExternalOutput").ap()

    with tile.TileContext(nc) as tc:
        with tc.tile_pool(name="const", bufs=1) as cpool, \
             tc.tile_pool(name="perb", bufs=1) as perb, \
             tc.tile_pool(name="stream", bufs=3) as stream, \
             tc.tile_pool(name="psum", bufs=1, space="PSUM") as pp:

            # ---- constants ----
            # gpsimd queue: QKV weights + small tensors (needed first)
            wq_sb = cpool.tile([128, D], W_DT, tag="wq")
            wk_sb = cpool.tile([128, D], W_DT, tag="wk")
            wv_sb = cpool.tile([128, D], W_DT, tag="wv")
            # gpsimd queue: only the three main weights; everything else on
            # the scalar queue (idle until emit_late_consts) so idm/dmask/
            # biases land early in parallel instead of serially behind them
            nc.gpsimd.dma_start(wq_sb[:], wq_d[:])
            nc.gpsimd.dma_start(wk_sb[:], wk_d[:])
            nc.gpsimd.dma_start(wv_sb[:], wv_d[:])
            bq_sb = cpool.tile([128, 1], F32, tag="bq")
            bk_sb = cpool.tile([128, 1], F32, tag="bk")
            bv_sb = cpool.tile([128, 1], F32, tag="bv")
            nc.scalar.dma_start(bq_sb[:], bq_d[:])
            nc.scalar.dma_start(bk_sb[:], bk_d[:])
            nc.scalar.dma_start(bv_sb[:], bv_d[:])
            id_sb = cpool.tile([128, 128], ID_DT, tag="idm")
            nc.scalar.dma_start(id_sb[:], id_d[:])
            if causal:
                dmask_sb = cpool.tile([128, 128], AT_DT, tag="dmask")
                nc.scalar.dma_start(dmask_sb[:], dmask_d[:])
            if USE_FP8 and QKV_COMP:
                wqr_sb = cpool.tile([128, D], W_DT, tag="wqr")
                wkr_sb = cpool.tile([128, D], W_DT, tag="wkr")
                wvr_sb = cpool.tile([128, D], W_DT, tag="wvr")
                nc.scalar.dma_start(wqr_sb[:], wqr_d[:])
                nc.scalar.dma_start(wkr_sb[:], wkr_d[:])
                nc.scalar.dma_start(wvr_sb[:], wvr_d[:])
            # scalar queue: output-projection constants - loaded after the
            # prologue QKV so their transfers don't delay the first x tiles
            wout_sb = cpool.tile([128, NDC * D], WOUT_DT, tag="wout")
            bout_sb = cpool.tile([1, D], F32R, tag="bout")
            ones_row = cpool.tile([1, 128], F32R, tag="onesr")

            def emit_late_consts():
                # chunked so no single transfer hogs the DMA engines
                for cc in range(NDC):
                    nc.scalar.dma_start(wout_sb[:, cc * D:(cc + 1) * D],
                                        wout_d[:, cc * D:(cc + 1) * D])
                nc.scalar.dma_start(bout_sb[:], bout_d[:])
                nc.scalar.dma_start(ones_row[:], onesr_d[:])

            def alloc_qkv():
                qt = perb.tile([128, T], QT_DT, tag="qt", bufs=2, name="qt")
                kt = perb.tile([128, T], KT_DT, tag="kt", bufs=2, name="kt")
                vt = perb.tile([128, T], VT_DT, tag="vt", bufs=2, name="vt")
                if USE_FP8:
                    # head h on partitions [32h,32h+32); dk = 32*i + pp
                    qt8 = perb.tile([64, 2, T], QT_DT, tag="qt8", bufs=2,
                                    name="qt8")
                    kt8 = perb.tile([64, 2, T], QT_DT, tag="kt8", bufs=2,
                                    name="kt8")
                    return qt, kt, vt, qt8, kt8
                return qt, kt, vt

            def emit_remap(qkv, tg):
                # partition fold [128,512] -> [64,2,512] per t-group via
                # SBUF->SBUF DMAs on the gpsimd queue (25ns dispatch vs 565
                # on SP); per-tg granularity keeps the remap off the
                # prologue critical path
                qt, kt = qkv[0], qkv[1]
                qt8, kt8 = qkv[3], qkv[4]
                s = slice(tg * 512, (tg + 1) * 512)
                for pre, packed in ((qt, qt8), (kt, kt8)):
                    for h in range(2):
                        for i in range(2):
                            nc.sync.dma_start(
                                packed[32 * h:32 * h + 32, i, s],
                                pre[64 * h + 32 * i:64 * h + 32 * i + 32, s])

            def emit_qkv_tg(qkv, b, tg):
                qt, kt, vt = qkv[0], qkv[1], qkv[2]
                c0 = b * T + tg * 512
                if USE_FP8:
                    xt = stream.tile([128, 4, 2, 512], XT_DT, tag="xt",
                                     bufs=XT_BUFS, name="xt")
                    nc.sync.dma_start(xt[:], xT_d[:, :, :, c0:c0 + 512])
                    if QKV_COMP:
                        xtr = stream.tile([128, 4, 2, 512], XT_DT, tag="xtr",
                                          bufs=XT_BUFS, name="xtr")
                        nc.sync.dma_start(xtr[:], xTr_d[:, :, :, c0:c0 + 512])
                else:
                    xt = stream.tile([128, NDC * 512], XT_DT, tag="xt",
                                     bufs=XT_BUFS, name="xt")
                    nc.sync.dma_start(
                        xt.rearrange("p (dc j) -> p dc j", j=512),
                        xT_d[:, :, c0:c0 + 512])
                if USE_FP8 and QKV_COMP:
                    wr_sbs = (wqr_sb, wkr_sb, wvr_sb)
                else:
                    wr_sbs = (None, None, None)
                for w_sb, wr_sb, bias_sb, dst in (
                        (wq_sb, wr_sbs[0], bq_sb, qt),
                        (wk_sb, wr_sbs[1], bk_sb, kt),
                        (wv_sb, wr_sbs[2], bv_sb, vt)):
                    ps = pp.tile([128, 512], F32, tag="mm", bufs=2, name="ps")
                    if USE_FP8:
                        wv8 = w_sb.rearrange("p (c2 i j) -> p c2 i j",
                                             c2=4, i=2)
                        chains = [(wv8, xt)]
                        if QKV_COMP:
                            wr8 = wr_sb.rearrange("p (c2 i j) -> p c2 i j",
                                                  c2=4, i=2)
                            chains += [(wv8, xtr), (wr8, xt)]
                        nch = len(chains)
                        for ci, (wch, xch) in enumerate(chains):
                            for c2 in range(4):
                                nc.tensor.matmul(
                                    ps[:], wch[:, c2], xch[:, c2],
                                    start=(ci == 0 and c2 == 0),
                                    stop=(ci == nch - 1 and c2 == 3),
                                    perf_mode=mybir.MatmulPerfMode.DoubleRow)
                    else:
                        for dc in range(NDC):
                            nc.tensor.matmul(
                                ps[:], w_sb[:, dc * 128:(dc + 1) * 128],
                                xt[:, dc * 512:(dc + 1) * 512],
                                start=(dc == 0), stop=(dc == NDC - 1))
                    nc.vector.tensor_scalar_add(
                        dst[:, tg * 512:(tg + 1) * 512], ps[:], bias_sb[:])

            def alloc_vnb():
                # vnb layout per k-chunk: [two heads][64 V rows + ones + pad]
                vnb = perb.tile([128, NKC * 132], VNB_DT, tag="vnb", bufs=2,
                                name="vnb")
                nc.vector.memset(
                    vnb.rearrange("p (c two w) -> p c two w", two=2, w=66)
                    [:, :, :, 64:65], 1.0)
                return vnb

            def emit_vchunks(vnb, vt, tg):
                for kc in range(4 * tg, 4 * tg + 4):
                    tp = pp.tile([128, 128], VT_DT, tag="mm", bufs=2, name="tp")
                    nc.tensor.transpose(tp[:], vt[:, kc * 128:(kc + 1) * 128],
                                        id_sb[:])
                    nc.vector.tensor_copy(
                        vnb.rearrange("p (c two w) -> p c two w", two=2, w=66)
                        [:, kc, :, 0:64],
                        tp[:].rearrange("p (two v) -> p two v", two=2))

            def emit_scores(qkv, qg, kc, live, diag):
                qt, kt = qkv[0], qkv[1]
                st = pp.tile([128, 1024], F32, tag="st", bufs=2, name="st")
                # diag blocks: start=True marks the whole 2KB region
                # pending-zero, so the span carrying it must come FIRST and
                # only once per head; later spans write start=False and are
                # zero-filled by the pending mechanism.  The checked
                # start/stop pair lives on one span so the exp read is
                # never mid-group; the rest skip the group check.
                # spans: (lo, hi, start, stop, skip)
                if diag:
                    if live + 128 < 512:
                        spans = [(live + 128, 512, True, True, False),
                                 (live, live + 128, False, False, True)]
                    else:
                        spans = [(live, live + 128, True, False, False)]
                else:
                    spans = [(0, 512, True, True, False)]
                if USE_FP8:
                    qt8, kt8 = qkv[3], qkv[4]
                    for h in range(2):
                        for lo, hi, srt, stp, skp in spans:
                            nc.tensor.matmul(
                                st[:, h * 512 + lo:h * 512 + hi],
                                kt8[32 * h:32 * h + 32, :,
                                    kc * 128:(kc + 1) * 128],
                                qt8[32 * h:32 * h + 32, :,
                                    qg * 512 + lo:qg * 512 + hi],
                                start=srt, stop=stp, skip_group_check=skp,
                                perf_mode=mybir.MatmulPerfMode.DoubleRow)
                else:
                    for h in range(2):
                        for lo, hi, srt, stp, skp in spans:
                            nc.tensor.matmul(
                                st[:, h * 512 + lo:h * 512 + hi],
                                kt[64 * h:64 * h + 64,
                                   kc * 128:(kc + 1) * 128],
                                qt[64 * h:64 * h + 64,
                                   qg * 512 + lo:qg * 512 + hi],
                                start=srt, stop=stp, skip_group_check=skp)
                if diag:
                    # triangular mask added on the PE itself: accumulating
                    # matmul id^T @ dmask == dmask, so the score->exp chain
                    # never leaves the tensor engine
                    for h in range(2):
                        nc.tensor.matmul(
                            st[:, h * 512 + live:h * 512 + live + 128],
                            id_sb[:], dmask_sb[:],
                            start=False,
                            stop=(live + 128 >= 512),
                            skip_group_check=(live + 128 < 512))
                elif not causal:
                    mt = stream.tile([128, 512], F32, tag="mt", bufs=3,
                                     name="mt")
                    nc.sync.dma_start(
                        mt[:], maskT_d[kc * 128:(kc + 1) * 128,
                                       qg * 512:(qg + 1) * 512])
                    nc.vector.tensor_add(st[:, 0:512], st[:, 0:512], mt[:])
                    nc.vector.tensor_add(st[:, 512:1024], st[:, 512:1024],
                                         mt[:])
                at = stream.tile([128, 1024], AT_DT, tag="at", bufs=6, name="at")
                esc = float(SCALE / SSC) if USE_FP8 else float(SCALE)
                nc.scalar.activation(
                    at.rearrange("p (two n) -> p two n", two=2)[:, :, live:512],
                    st.rearrange("p (two n) -> p two n", two=2)[:, :, live:512],
                    Exp, scale=esc)
                return at

            def emit_pv(acc, vnb, at, qg, kc, o):
                # q-partitioned PV: out [128 q, 65] per (head, q-chunk);
                # stationary = the at block [128 k, 128 q], moving = vnb
                # [128 k, 65].  Cost is 65 moving columns instead of 512.
                # Each q-chunk's accumulation group closes at its own
                # diagonal k-chunk.
                # start=True lazily marks the WHOLE 2KB psum bank region as
                # pending-zero, so only the first matmul of each head's
                # region may carry it; sibling slots' first writes then
                # zero-fill via the pending mechanism
                qc0 = max(o, 0) if causal else 0
                for h in range(2):
                    for qc in range(qc0, 4):
                        lastk = (4 * qg + qc) if causal else (NKC - 1)
                        nc.tensor.matmul(
                            acc[:, h * 4 + qc, 0:65],
                            at[:, h * 512 + qc * 128:h * 512 + qc * 128 + 128],
                            vnb[:, kc * 132 + 66 * h:kc * 132 + 66 * h + 65],
                            start=(kc == 0 and qc == 0),
                            stop=(kc == lastk),
                            skip_group_check=True)

            def make_close(qg, acc, ytss, tail=False):  # noqa: tail unused
                # close1: per-partition normalize straight out of PSUM
                # (DVE-only, so the next q-group's PV unblocks quickly);
                # close2 (one chunk later): PE transpose to Y^T orientation
                # with a permuted out AP so the yts shuffle-copies are
                # packed-bf16 (DVE 2x), then copy into yts
                nsbs = {}

                def mk_norm(h, qp):
                    # normalize q-chunk pair (2qp, 2qp+1) of head h; qp=0's
                    # accumulators stop at their early diagonal blocks, so
                    # it can run a chunk before the q-group ends (causal),
                    # halving the DVE chain that gates the next PV
                    def norm():
                        if qp == 0:
                            # bufs=4: delayed close2 parts (up to 6 flushes
                            # out) must still see this close's data after
                            # the next close allocates fresh buffers
                            nsbs[h] = stream.tile([128, 4, 64], BF,
                                                  tag="nsb", bufs=4,
                                                  name="nsb")
                        nsb = nsbs[h]
                        s0 = h * 4 + 2 * qp
                        rcp = stream.tile([128, 2], F32, tag="rcp",
                                          bufs=4, name="rcp")
                        nc.vector.reciprocal(
                            rcp[:], acc[:, s0:s0 + 2, 64:65])
                        for i in range(2):
                            nc.vector.tensor_scalar_mul(
                                nsb[:, 2 * qp + i], acc[:, s0 + i, 0:64],
                                rcp[:, i:i + 1])
                    return norm

                def close1():
                    for h in range(2):
                        nsbs[h] = stream.tile([128, 4, 64], BF, tag="nsb",
                                              bufs=4, name="nsb")
                        rcp = stream.tile([128, 4], F32, tag="rcp",
                                          bufs=4, name="rcp")
                        nc.vector.reciprocal(
                            rcp[:], acc[:, h * 4:h * 4 + 4, 64:65])
                        for qc in range(4):
                            nc.vector.tensor_scalar_mul(
                                nsbs[h][:, qc], acc[:, h * 4 + qc, 0:64],
                                rcp[:, qc:qc + 1])

                early = []

                def mk_close2(h, qp):
                    def close2():
                        nsb = nsbs[h]
                        ytv = ytss[h].rearrange("p (c r) -> p c r", r=128)
                        # two q-chunks per transpose: in [128 q, 2, 64]
                        # -> out partitions (qc, v), columns q (step-1 out
                        # AP; hw requires innermost step 1 on transpose)
                        tp2 = pp.tile([128, 128], BF, tag="mm", bufs=2,
                                      name="tp2")
                        nc.tensor.transpose(
                            tp2[:], nsb[:, 2 * qp:2 * qp + 2], id_sb[:])
                        tv = tp2[:].rearrange(
                            "v (rl j two) -> v two j rl", two=2, j=8)
                        for qi in range(2):
                            r0 = qg * 32 + (2 * qp + qi) * 8
                            for pi in range(2):
                                nc.vector.tensor_copy(
                                    ytv[64 * pi:64 * pi + 64, :,
                                        r0:r0 + 8],
                                    tv[64 * qi:64 * qi + 64, pi])
                    return close2

                return close1, [mk_close2(h, qp)
                                for h in range(2) for qp in range(2)], early

            pend_store = {0: None, 1: None}

            def flush_store(h):
                if pend_store[h] is not None:
                    dst, ost = pend_store[h]
                    nc.sync.dma_start(dst, ost[:])
                    pend_store[h] = None

            def emit_phase5(b, h, dt2s, ytss, final=False):
                dt2 = dt2s[h]
                yts = ytss[h]
                flush_store(h)
                ost = stream.tile([128, D], F32, tag="ost", bufs=2, name="ost")
                for og in range(2):
                    po = pp.tile([128, 512], F32, tag="mm", bufs=2, name="po")
                    if OUT_FP8:
                        # DoubleRow pairs adjacent 128-col chunks: yts
                        # [p, (c r)] viewed as [p, c2, i, r] and wout_sb
                        # [p, (dc j)] as [p, c2, i, j] pair d-chunks
                        # (2*c2, 2*c2+1) identically - pure views, no remap
                        yt8 = yts.rearrange("p (c2 i r) -> p c2 i r",
                                            i=2, r=128)
                        wo8 = wout_sb.rearrange("p (c2 i j) -> p c2 i j",
                                                i=2, j=D)
                        for c2 in range(NDC // 2):
                            nc.tensor.matmul(
                                po[:], yt8[:, c2],
                                wo8[:, c2, :, og * 512:og * 512 + 512],
                                start=(c2 == 0), stop=False,
                                perf_mode=mybir.MatmulPerfMode.DoubleRow)
                    else:
                        for cc in range(NDC):
                            nc.tensor.matmul(
                                po[:], yts[:, cc * 128:(cc + 1) * 128],
                                wout_sb[:, cc * D + og * 512:cc * D + og * 512 + 512],
                                start=(cc == 0), stop=False)
                    nc.tensor.matmul(po[:], ones_row[:],
                                     bout_sb[0:1, og * 512:(og + 1) * 512],
                                     start=False, stop=True)
                    nc.vector.tensor_mul(
                        ost[:, og * 512:(og + 1) * 512], po[:],
                        dt2[:, og * 512:(og + 1) * 512])
                    if final:
                        nc.sync.dma_start(
                            out_d[b, h * 128:(h + 1) * 128,
                                  og * 512:(og + 1) * 512],
                            ost[:, og * 512:(og + 1) * 512])
                if not final:
                    pend_store[h] = (out_d[b, h * 128:(h + 1) * 128, :], ost)

            # PE p-state warmup: self-contained matmul chain (no DMA deps)
            # keeps the clock ramp alive while the first x tiles stream in
            warm_sb = cpool.tile([128, 512], BF, tag="warm")
            nc.vector.memset(warm_sb[:], 0.0)

            def emit_warmup(n):
                wp = pp.tile([128, 512], F32, tag="mm", bufs=2, name="wp")
                for i in range(n):
                    nc.tensor.matmul(wp[:], warm_sb[:, 0:128], warm_sb[:],
                                     start=True, stop=True)

            # ============ flat software-pipelined schedule ============
            # Each chunk's PV (and, on the last chunk of a q-group, the
            # evict/norm close plus the boundary fillers: next batch's QKV,
            # V-transposes, previous batch's output projection) is emitted
            # only AFTER the next chunk's scores, so the PE stream never
            # drains waiting on exp or the accumulator evict.
            def emit_qkv_full(qkv, b, tg):
                emit_qkv_tg(qkv, b, tg)
                if USE_FP8:
                    emit_remap(qkv, tg)

            # attention q-group qg only needs t-groups <= qg of Q/K and
            # V chunks <= qg, so b0's tg2/tg3 become boundary fillers
            # inside its own early attention instead of serial prologue
            cur = alloc_qkv()
            cur_vnb = alloc_vnb()
            emit_warmup(8)
            emit_qkv_full(cur, 0, 0)
            emit_qkv_full(cur, 0, 1)
            emit_vchunks(cur_vnb, cur[2], 0)

            carry = {"v": None}  # pending PV (+close/fillers) of prev chunk
            pend_posts = []      # (countdown, fn): close2 deferred 3 flushes
            pend_vchunks = []    # deferred tg3 V-transposes of the next batch

            def flush_posts(force=False):
                rest = []
                for cnt, fn in pend_posts:
                    if force or cnt <= 0:
                        fn()
                    else:
                        rest.append((cnt - 1, fn))
                pend_posts[:] = rest

            def flush_carry():
                flush_posts()
                c = carry["v"]
                if c is None:
                    return
                emit_pv(*c["pv"])
                for fn in c.get("extras", []):
                    fn()
                if c["close"] is not None:
                    c["close"]()
                    for i, fn in enumerate(c["close2"]):
                        pend_posts.append((3 + i, fn))
                for f in c["fillers"]:
                    f()
                carry["v"] = None

            prev = None   # (b-1's dt2s, ytss) - phase5 runs inside attn(b)
            for b in range(B):
                ytss = [stream.tile([128, NDC * 128], YT_DT, tag="yt",
                                    bufs=4, name="yts") for _ in range(HP)]
                if b + 1 < B:
                    nxt = alloc_qkv()
                    nxt_vnb = alloc_vnb()
                dt2s = []
                for qg in range(NQG):
                    kcmax = 4 * qg + 4 if causal else NKC
                    # PV accumulators: 8 slots (2 heads x 4 q-chunks) of
                    # [128 q, 65] at 512B stride - 2 PSUM banks
                    acc = pp.tile([128, 8, 128], F32, tag="cs", bufs=1,
                                  name="acc")
                    fillers = []
                    if qg == 0 and pend_vchunks:
                        fillers.extend(pend_vchunks)
                        pend_vchunks.clear()
                    if b == 0:
                        if qg < 2:
                            fillers.append(
                                lambda cc=cur, tg=qg + 2:
                                    emit_qkv_full(cc, 0, tg))
                            fillers.append(
                                lambda vb=cur_vnb, vt=cur[2], tg=qg + 1:
                                    emit_vchunks(vb, vt, tg))
                        elif qg == 2:
                            fillers.append(
                                lambda vb=cur_vnb, vt=cur[2]:
                                    emit_vchunks(vb, vt, 3))
                    if b == 0 and qg == 1:
                        fillers.append(emit_late_consts)
                    if b + 1 < B:
                        fillers.append(
                            lambda nx=nxt, bb=b + 1, tg=qg:
                                emit_qkv_full(nx, bb, tg))
                        if qg > 0:
                            fillers.append(
                                lambda vb=nxt_vnb, vt=nxt[2], tg=qg - 1:
                                    emit_vchunks(vb, vt, tg))
                    p5qg = qg - 1
                    if prev is not None and 0 <= p5qg < HP:
                        fillers.append(
                            lambda bb=b - 1, h=p5qg, pv=prev:
                                emit_phase5(bb, h, *pv))
                    if qg == NQG - 1:
                        def _dt(b=b, dt2s=dt2s):
                            for h in range(HP):
                                dt2 = stream.tile([128, D], BF, tag="dt",
                                                  bufs=2, name="dt2")
                                nc.sync.dma_start(
                                    dt2[:],
                                    drop_d[b, h * 128:(h + 1) * 128, :])
                                dt2s.append(dt2)
                        fillers.append(_dt)
                        if b + 1 < B:
                            pend_vchunks.append(
                                lambda vb=nxt_vnb, vt=nxt[2]:
                                    emit_vchunks(vb, vt, NTG - 1))
                    close1, close2, early = make_close(qg, acc, ytss,
                                                       tail=(b == B - 1))
                    for kc in range(kcmax):
                        o = kc - 4 * qg
                        diag = causal and o >= 0
                        live = o * 128 if diag else 0
                        at = emit_scores(cur, qg, kc, live, diag)
                        flush_carry()
                        last = (kc == kcmax - 1)
                        carry["v"] = {
                            "pv": (acc, cur_vnb, at, qg, kc, o),
                            "close": close1 if last else None,
                            "close2": close2 if last else [],
                            # early norms: q-chunks 0/1 stopped at their
                            # diagonal blocks two chunks ago
                            "extras": early if (causal and
                                                kc == kcmax - 2) else [],
                            "fillers": fillers if last else [],
                        }
                prev = (dt2s, ytss)
                if b + 1 < B:
                    cur, cur_vnb = nxt, nxt_vnb
            flush_carry()
            flush_posts(force=True)
            for h in range(HP):
                emit_phase5(B - 1, h, *prev, final=True)
            flush_store(0)
            flush_store(1)

    nc.compile()
    return nc


def _get_program(causal: bool):
    key = ("causal" if causal else "full")
    if key not in _cache:
        _cache[key] = _build(causal)
    return _cache[key]


def _host_fallback(x, attn_mask, Wq, bq, Wk, bk, Wv, bv, Wout, bout,
                   dropout_mask):
    x64 = x.astype(np.float32)
    Q = np.einsum("btd,hdk->bhtk", x64, Wq) + bq[None, :, None, :]
    K = np.einsum("btd,hdk->bhtk", x64, Wk) + bk[None, :, None, :]
    V = np.einsum("btd,hdv->bhtv", x64, Wv) + bv[None, :, None, :]
    scores = np.einsum("bhqk,bhmk->bhqm", Q, K) * SCALE + attn_mask
    scores = scores - scores.max(-1, keepdims=True)
    e = np.exp(scores)
    attn = e / e.sum(-1, keepdims=True)
    ctx = np.einsum("bhqm,bhmv->bhqv", attn, V).reshape(B, T, H * DV)
    out = ctx @ Wout.T + bout
    return (out * dropout_mask).astype(np.float32)


def _chunked128(w):
    """[D, M] -> [128, (dc M)] with row d = dc*128 + p."""
    d, m = w.shape
    return np.ascontiguousarray(
        w.reshape(d // 128, 128, m).transpose(1, 0, 2).reshape(128, -1))


def kernel(x, attn_mask, Wq, bq, Wk, bk, Wv, bv, Wout, bout, dropout_mask):
    from concourse.bass_utils import run_bass_kernel_spmd

    x = np.ascontiguousarray(x, np.float32)
    m = np.asarray(attn_mask, np.float32).reshape(T, T)

    # causality check on the actual mask tensor
    causal = bool((np.tril(m) == 0).all() and
                  (m[np.triu_indices(T, 1)] <= -1e8).all())

    # safety: cheap bound on max |scaled score| -> exp overflow guard
    xf = x.reshape(B * T, D)
    Qa = xf @ Wq.transpose(1, 0, 2).reshape(D, H * DK)
    Ka = xf @ Wk.transpose(1, 0, 2).reshape(D, H * DK)
    Qa = Qa.reshape(B * T, H, DK) + bq[None]
    Ka = Ka.reshape(B * T, H, DK) + bk[None]
    qn = np.linalg.norm(Qa, axis=2).max(0)     # per-head max row norm
    kn = np.linalg.norm(Ka, axis=2).max(0)
    bound = float(SCALE) * float((qn * kn).max())
    if bound > 50.0:
        return _host_fallback(x, attn_mask, Wq, bq, Wk, bk, Wv, bv, Wout,
                              bout, dropout_mask)

    nc = _get_program(causal)

    xd = x.transpose(2, 0, 1).reshape(D, B * T)
    xTr = None
    if USE_FP8:
        # [128, c2, i, B*T] with d = c2*256 + 2p + i
        def _xlay(a):
            return np.ascontiguousarray(
                a.reshape(4, 128, 2, B * T).transpose(1, 0, 2, 3))
        x8 = xd.astype(FP8E4)
        xT = _xlay(x8)
        if QKV_COMP:
            xTr = _xlay((xd - x8.astype(np.float32)).astype(FP8E4))
    else:
        xT = np.ascontiguousarray(
            xd.reshape(NDC, 128, B * T).transpose(1, 0, 2)
        ).astype(np.float32 if XT_F32R else BF16)
    woutT = np.asarray(Wout, np.float32).T            # [f, o]
    if OUT_FP8:
        wout_sb = _chunked128(woutT * np.float32(WSC)).astype(FP8E4)
    else:
        wout_sb = _chunked128(woutT).astype(np.float32 if WOUT_F32R else BF16)
    boutr = np.asarray(bout, np.float32).reshape(1, D) * np.float32(OSC)
    _w_np = np.float32 if XT_F32R else BF16

    def _wpack(w):   # [D, 128] -> [128, D] main + fp8 residual packs
        if USE_FP8:
            def lay(a):
                return np.ascontiguousarray(
                    a.reshape(4, 128, 2, 128)
                    .transpose(1, 0, 2, 3).reshape(128, D))
            ws = np.asarray(w, np.float32) * np.float32(WSC)
            w8 = ws.astype(FP8E4)
            wr = (ws - w8.astype(np.float32)).astype(FP8E4)
            return lay(w8), lay(wr)
        return _chunked128(w).astype(_w_np), None
    idm = np.eye(128, dtype=np.float32) if AT_F32R else np.eye(128, dtype=np.float32).astype(BF16)
    _ssc = np.float32(SSC) if USE_FP8 else np.float32(1.0)
    dmask = np.where(np.arange(128)[None, :] < np.arange(128)[:, None],
                     MASK_NEG * _ssc, np.float32(0.0)).astype(
                         np.float32 if AT_F32R else BF16)
    maskT = None if causal else np.ascontiguousarray(m.T * np.float32(8.0) * _ssc)
    drop = (np.asarray(dropout_mask, np.float32) / np.float32(OSC)).astype(BF16)

    in_maps = []
    for c in range(NCORES):
        h0, h1 = HP * c, HP * c + 1
        wq8, wqr = _wpack(np.concatenate([Wq[h0], Wq[h1]], axis=1))
        wk8, wkr = _wpack(np.concatenate([Wk[h0], Wk[h1]], axis=1))
        wv8, wvr = _wpack(np.concatenate([Wv[h0], Wv[h1]], axis=1))
        im = {
            "xT": xT,
            "wq": wq8,
            "wk": wk8,
            "wv": wv8,
            "bq": np.concatenate([bq[h0], bq[h1]]).reshape(128, 1)
                    .astype(np.float32) * np.float32(YSC),
            "bk": np.concatenate([bk[h0], bk[h1]]).reshape(128, 1)
                    .astype(np.float32) * np.float32(YSC),
            "bv": np.concatenate([bv[h0], bv[h1]]).reshape(128, 1)
                    .astype(np.float32) * np.float32(YSC),
            "wout": wout_sb,
            "bout": boutr,
            "onesr": np.ones((1, 128), np.float32),
            "drop": np.ascontiguousarray(drop[:, c * ROWS:(c + 1) * ROWS, :]),
            "idm": idm,
        }
        if USE_FP8 and QKV_COMP:
            im["xTr"] = xTr
            im["wqr"] = wqr
            im["wkr"] = wkr
            im["wvr"] = wvr
        if causal:
            im["dmask"] = dmask
        else:
            im["maskT"] = maskT
        in_maps.append(im)

    res = run_bass_kernel_spmd(nc, in_maps, list(range(NCORES)))
    out = np.empty((B, T, D), np.float32)
    for c in range(NCORES):
        out[:, c * ROWS:(c + 1) * ROWS, :] = res.results[c]["out"]
    return out

